# revision 1
# baseline (speedup 1.0000x reference)
"""Causal GQA attention (B=4, S=1024, H=16 q-heads, 4 kv-heads, D=128) on 8 trn2 cores.

Sharding: 16 (batch, kv-group) pairs -> 2 pairs/core; each pair carries 4 query
heads, so each core runs 8 independent causal-attention head-units.

Per head-unit math (all on one core):
  S^T[sk, sq] = K^T.T @ Q^T          (contraction over d=128 on partitions)
  P^T = exp(SCALE * S^T)             (ACT, fused scale; no max-subtraction --
                                      scores ~ N(0,1) so exp never overflows)
  diagonal 128x128 block masked with a 0/1 upper-tri mask (DVE multiply)
  O[sq, 0:128], den[sq] = P^T.T @ [V | 1]   (denominator free in column 128)
  out = O * (1/den)                  (DVE reciprocal + per-partition scale)

Head-units are software-pipelined (scores/exp of head u overlap PV of head
u-1) so the ACT exp stream and the PE matmul stream run concurrently.
"""

import os
import sys

for _p in ("/opt/trn_rl_repo", "/root/.axon_site/_ro/trn_rl_repo"):
    if os.path.isdir(_p) and _p not in sys.path:
        sys.path.insert(0, _p)

from contextlib import ExitStack

import numpy as np

import concourse.bass as bass
import concourse.tile as tile
from concourse import bacc, mybir
from concourse.bass_utils import run_bass_kernel_spmd

B = 4
S = 1024
H = 16
HKV = 4
G = H // HKV  # 4 query heads per kv head
D = 128
SCALE = 0.08838834764831845
NCORES = 8
PAIRS_PER_CORE = (B * HKV) // NCORES  # 2
NU = PAIRS_PER_CORE * G  # 8 head-units per core
NT = S // 128  # 8 tiles of 128 along seq
VW = D + 1  # V columns + ones column (fused softmax denominator)

FP16 = mybir.dt.float16
FP32 = mybir.dt.float32

_cache = {}


def build_program(n_units=NU):
    nc = bacc.Bacc("TRN2", target_bir_lowering=False, debug=False, num_devices=NCORES)

    qt_d = nc.dram_tensor("qt", [NU, D, S], FP16, kind="ExternalInput").ap()
    kt_d = nc.dram_tensor("kt", [PAIRS_PER_CORE, D, S], FP16, kind="ExternalInput").ap()
    vp_d = nc.dram_tensor("vp", [PAIRS_PER_CORE, NT, 128, VW], FP16, kind="ExternalInput").ap()
    mask_d = nc.dram_tensor("mask", [128, 128], FP16, kind="ExternalInput").ap()
    o_d = nc.dram_tensor("o", [PAIRS_PER_CORE, S, G, D], FP32, kind="ExternalOutput").ap()

    with tile.TileContext(nc) as tc, ExitStack() as ctx:
        const = ctx.enter_context(tc.tile_pool(name="const", bufs=1))
        pt_pool = ctx.enter_context(tc.tile_pool(name="pt_pool", bufs=2))
        small = ctx.enter_context(tc.tile_pool(name="small", bufs=4))
        outp = ctx.enter_context(tc.tile_pool(name="outp", bufs=2))
        psum = ctx.enter_context(tc.tile_pool(name="psum", bufs=2, space="PSUM"))

        # ---- loads, ordered so head 0 can start as early as possible ----
        kt_sb = const.tile([128, PAIRS_PER_CORE, S], FP16)
        qt_sb = const.tile([128, NU, S], FP16)
        vp_sb = const.tile([128, PAIRS_PER_CORE * NT, VW], FP16)
        mask_sb = const.tile([128, 128], FP16)

        nc.sync.dma_start(out=kt_sb[:, 0, 0:128], in_=kt_d[0][:, 0:128])
        nc.sync.dma_start(out=qt_sb[:, 0, :], in_=qt_d[0])
        nc.sync.dma_start(out=kt_sb[:, 0, 128:S], in_=kt_d[0][:, 128:S])
        nc.sync.dma_start(out=mask_sb, in_=mask_d)
        nc.sync.dma_start(
            out=vp_sb[:, 0:NT, :], in_=vp_d[0].rearrange("j r c -> r j c")
        )
        for u in range(1, G):
            nc.sync.dma_start(out=qt_sb[:, u, :], in_=qt_d[u])
        nc.sync.dma_start(out=kt_sb[:, 1, :], in_=kt_d[1])
        nc.sync.dma_start(
            out=vp_sb[:, NT : 2 * NT, :], in_=vp_d[1].rearrange("j r c -> r j c")
        )
        for u in range(G, NU):
            nc.sync.dma_start(out=qt_sb[:, u, :], in_=qt_d[u])

        def head(u):
            pair, h = divmod(u, G)
            pt = pt_pool.tile([128, NT, S], FP16, tag="pt", name=f"pt_{u}")
            ob = outp.tile([128, NT, D], FP32, tag="ot", name=f"ot_{u}")
            def pv_tile(i):
                po = psum.tile([128, VW], FP32, tag="pv", name=f"pv_{u}_{i}")
                for jj in range(i + 1):
                    nc.tensor.matmul(
                        po,
                        lhsT=pt[:, jj, 128 * i : 128 * i + 128],
                        rhs=vp_sb[:, pair * NT + jj, :],
                        start=(jj == 0),
                        stop=(jj == i),
                    )
                rec = small.tile([128, 1], FP32, tag="rec", name=f"rec_{u}_{i}")
                nc.vector.reciprocal_approx_fast(rec, po[:, D : D + 1])
                nc.vector.tensor_scalar_mul(ob[:, i, :], po[:, 0:D], rec)

            # rows 0..3 individually (wide); rows (4,5) and (6,7) paired into
            # one PSUM super-tile + ONE exp call each (amortizes ACT per-call
            # overhead; the paired rows' extra sub-diagonal columns are real
            # finite scores that PV never reads, so no masking needed there)
            for j in range(4):
                sq0 = 128 * j
                w = S - sq0
                ps = psum.tile([128, 1024], FP32, tag="ps2", name=f"ps2_{u}_{j}")
                lhsT = kt_sb[:, pair, sq0 : sq0 + 128]
                for c0 in range(0, w, 512):
                    cw = min(512, w - c0)
                    nc.tensor.matmul(
                        ps[:, c0 : c0 + cw],
                        lhsT=lhsT,
                        rhs=qt_sb[:, u, sq0 + c0 : sq0 + c0 + cw],
                        start=True,
                        stop=True,
                    )
                nc.scalar.activation(
                    out=pt[:, j, sq0:S],
                    in_=ps[:, 0:w],
                    func=mybir.ActivationFunctionType.Exp,
                    scale=SCALE,
                )
                nc.vector.tensor_mul(
                    pt[:, j, sq0 : sq0 + 128], pt[:, j, sq0 : sq0 + 128], mask_sb
                )
                pv_tile(j)
            for j0, wp, tag in ((4, 512, "ps45"), (6, 256, "ps67")):
                base = S - wp
                ps = psum.tile([128, 2, 512], FP32, tag="psp", name=f"{tag}_{u}", bufs=1)[:, :, 0:wp]
                for r in range(2):
                    j = j0 + r
                    lhsT = kt_sb[:, pair, 128 * j : 128 * j + 128]
                    nc.tensor.matmul(
                        ps[:, r, :],
                        lhsT=lhsT,
                        rhs=qt_sb[:, u, base:S],
                        start=True,
                        stop=True,
                    )
                nc.scalar.activation(
                    out=pt[:, j0 : j0 + 2, base:S],
                    in_=ps,
                    func=mybir.ActivationFunctionType.Exp,
                    scale=SCALE,
                )
                for r in range(2):
                    j = j0 + r
                    sq0 = 128 * j
                    nc.vector.tensor_mul(
                        pt[:, j, sq0 : sq0 + 128], pt[:, j, sq0 : sq0 + 128], mask_sb
                    )
                    pv_tile(j)
            # one batched store per head: [s-in-tile, i, d] -> o[pair, 128i+s, h, d]
            nc.sync.dma_start(
                out=o_d[pair, :, h, :].rearrange("(i s) d -> s i d", s=128), in_=ob
            )

        for u in range(n_units):
            head(u)

    nc.compile()
    return nc


def _host_prep(q, k, v):
    """Build per-core input maps (shard + transpose + fp16 cast on host)."""
    q16 = np.ascontiguousarray(q.astype(np.float16))
    k16 = np.ascontiguousarray(k.astype(np.float16))
    v16 = np.ascontiguousarray(v.astype(np.float16))

    ii = np.arange(128)
    mask = (ii[None, :] >= ii[:, None]).astype(np.float16)  # [jj, ii]: ii >= jj

    in_maps = []
    for c in range(NCORES):
        qt = np.empty((NU, D, S), np.float16)
        kt = np.empty((PAIRS_PER_CORE, D, S), np.float16)
        vp = np.empty((PAIRS_PER_CORE, NT, 128, VW), np.float16)
        for p in range(PAIRS_PER_CORE):
            pg = c * PAIRS_PER_CORE + p
            b, g = divmod(pg, HKV)
            tok = slice(b * S, (b + 1) * S)
            for hh in range(G):
                qt[p * G + hh] = q16[tok, g * G + hh, :].T
            kt[p] = k16[tok, g, :].T
            vseg = v16[tok, g, :]  # [S, D]
            vp[p, :, :, :D] = vseg.reshape(NT, 128, D)
            vp[p, :, :, D] = np.float16(1.0)
        in_maps.append({"qt": qt, "kt": kt, "vp": vp, "mask": mask})
    return in_maps


def _gather(results):
    out = np.empty((B * S, H, D), np.float32)
    for c in range(NCORES):
        o = results[c]["o"]  # [PAIRS, S, G, D]
        for p in range(PAIRS_PER_CORE):
            pg = c * PAIRS_PER_CORE + p
            b, g = divmod(pg, HKV)
            out[b * S : (b + 1) * S, g * G : (g + 1) * G, :] = o[p]
    return out


def kernel(q, k, v, cu_seqlens_q=None, cu_seqlens_k=None, **_ignored):
    if "nc" not in _cache:
        _cache["nc"] = build_program()
    nc = _cache["nc"]

    in_maps = _host_prep(np.asarray(q), np.asarray(k), np.asarray(v))
    res = run_bass_kernel_spmd(nc, in_maps, core_ids=list(range(NCORES)))
    return _gather(res.results)



# revision 4
# speedup vs baseline: 1.2817x; 1.2817x over previous
"""Causal GQA attention (B=4, S=1024, H=16 q-heads, 4 kv-heads, D=128) on 8 trn2 cores.

Sharding: 16 (batch, kv-group) pairs -> 2 pairs/core; each pair carries 4 query
heads, so each core runs 8 independent causal-attention head-units.

Engine plan per head-unit (all on one core):
  PE:  scores S^T[sk,sq] = K^T.T @ Q^T in fp16 (1 cyc/col; fp8 DoubleRow was
       tried and fails the 2e-2 budget: concentrated-attention rows amplify
       score noise by w*(1-w)*|V-O| ~ 0.1).  PV in fp16 with a fused ones
       column for the softmax denominator.  No mask work on PE.
  ACT: exact exp (fp16 out) for score chunks A (rows 0-2), C, D, E.
  DVE: Schraudolph exp2 for chunk B only: int16 bits = y*A16+B16 bitcast
       fp16 (~3% relerr, fine for windows >= 385), diagonal mask fused as
       (y + B16/A16) * (A16*mask) -> +0; batched reciprocal_approx_fast on
       the fused denominators; one broadcast-multiply per PV group for the
       softmax normalize (GPSIMD cannot touch PSUM on HW).
  Pool: 0/1 mask multiplies on the diagonal blocks of ACT chunks (SBUF).

Score region = 36 [128x128] causal blocks (k-block j, q-block i), packed
i-major into five row-aligned PSUM chunks (<=1024 cols, 2 banks, bufs=3)
so the PE/ACT/DVE pipeline has slack; pt chunks mirror the packing in SBUF.
"""

import os
import sys

for _p in ("/opt/trn_rl_repo", "/root/.axon_site/_ro/trn_rl_repo"):
    if os.path.isdir(_p) and _p not in sys.path:
        sys.path.insert(0, _p)

from contextlib import ExitStack

import numpy as np

import concourse.bass as bass
import concourse.tile as tile
from concourse import bacc, mybir
from concourse.bass_utils import run_bass_kernel_spmd

B = 4
S = 1024
H = 16
HKV = 4
G = H // HKV
D = 128
SCALE = 0.08838834764831845
LOG2E = 1.4426950408889634
NCORES = 8
PAIRS_PER_CORE = (B * HKV) // NCORES  # 2
NU = PAIRS_PER_CORE * G  # 8 head-units per core
NT = S // 128  # 8 k-blocks
VW = D + 1  # V columns + ones column (fused denominator)

A16 = 1024.0 * SCALE * LOG2E
B16 = 15 * 1024.0 - 44.45 + 0.5  # exp2 interp centering + trunc bias

FP16 = mybir.dt.float16
FP32 = mybir.dt.float32
I16 = mybir.dt.int16
EXP = mybir.ActivationFunctionType.Exp
ADD = mybir.AluOpType.add
MULT = mybir.AluOpType.mult
DIV = mybir.AluOpType.divide
BYPASS = mybir.AluOpType.bypass

# ---- static chunk tables: i-major, row-aligned, <=8 blocks per chunk ----
CHUNK_BLOCKS = [
    [(0, 0), (1, 1), (2, 2), (0, 1), (0, 2), (1, 2)],                  # A rows 0-2
    [(0, 3), (1, 3), (2, 3), (3, 3), (0, 4), (1, 4), (2, 4), (3, 4)],  # B
    [(4, 4), (5, 5), (0, 5), (1, 5), (2, 5), (3, 5), (4, 5)],          # C
    [(6, 6), (0, 6), (1, 6), (2, 6), (3, 6), (4, 6), (5, 6)],          # D
    [(7, 7), (0, 7), (1, 7), (2, 7), (3, 7), (4, 7), (5, 7), (6, 7)],  # E
]
# diagonal blocks are packed first in each ACT chunk -> single batched mask mul
DIAG_W = [3 * 128, 0, 2 * 128, 128, 128]
CHUNK_ENGINE = ["ACT", "DVE", "ACT", "ACT", "ACT"]
BLOCK_POS = {}
for _c, _blks in enumerate(CHUNK_BLOCKS):
    for _idx, _b in enumerate(_blks):
        BLOCK_POS[_b] = (_c, 128 * _idx)
DVE_CHUNKS = [c for c, e in enumerate(CHUNK_ENGINE) if e == "DVE"]
MA_W = sum(128 * len(CHUNK_BLOCKS[c]) for c in DVE_CHUNKS)

_cache = {}


def build_program():
    nc = bacc.Bacc("TRN2", target_bir_lowering=False, debug=False, num_devices=NCORES)

    qt_d = nc.dram_tensor("qt", [NU, D, S], FP16, kind="ExternalInput").ap()
    kt_d = nc.dram_tensor("kt", [PAIRS_PER_CORE, D, S], FP16, kind="ExternalInput").ap()
    vp_d = nc.dram_tensor("vp", [PAIRS_PER_CORE, NT, 128, VW], FP16, kind="ExternalInput").ap()
    ma_d = nc.dram_tensor("ma", [128, MA_W], FP16, kind="ExternalInput").ap()
    dm_d = nc.dram_tensor("dm", [128, 384], FP16, kind="ExternalInput").ap()
    o_d = nc.dram_tensor("o", [NU, 128, NT, D], FP16, kind="ExternalOutput").ap()

    with tile.TileContext(nc) as tc, ExitStack() as ctx:
        const = ctx.enter_context(tc.tile_pool(name="const", bufs=1))
        ptp = ctx.enter_context(tc.tile_pool(name="ptp", bufs=2))
        obp = ctx.enter_context(tc.tile_pool(name="obp", bufs=2))
        recp = ctx.enter_context(tc.tile_pool(name="recp", bufs=3))
        scp = ctx.enter_context(tc.tile_pool(name="scp", bufs=3, space="PSUM"))
        pvp = ctx.enter_context(tc.tile_pool(name="pvp", bufs=2, space="PSUM"))

        kt_sb = const.tile([128, PAIRS_PER_CORE, S], FP16)
        qt_sb = const.tile([128, NU, S], FP16)
        vp_sb = const.tile([128, PAIRS_PER_CORE * NT, VW], FP16)
        ma_sb = const.tile([128, MA_W], FP16)
        dm_sb = const.tile([128, 384], FP16)

        # loads: chunk-A slices (rows/k-blocks 0-2) first, issued in parallel
        # from SP and ACT sequencers so head 0's matmuls start ~2.5us in
        nc.sync.dma_start(out=kt_sb[:, 0, 0:384], in_=kt_d[0][:, 0:384])
        nc.scalar.dma_start(out=qt_sb[:, 0, 0:384], in_=qt_d[0][:, 0:384])
        nc.gpsimd.dma_start(out=ma_sb, in_=ma_d)
        nc.sync.dma_start(out=kt_sb[:, 0, 384:S], in_=kt_d[0][:, 384:S])
        nc.scalar.dma_start(out=qt_sb[:, 0, 384:S], in_=qt_d[0][:, 384:S])
        nc.gpsimd.dma_start(out=dm_sb, in_=dm_d)
        nc.sync.dma_start(out=qt_sb[:, 1, :], in_=qt_d[1])
        nc.scalar.dma_start(out=vp_sb[:, 0:NT, :], in_=vp_d[0].rearrange("j r c -> r j c"))
        for u in range(2, G):
            nc.sync.dma_start(out=qt_sb[:, u, :], in_=qt_d[u])
        nc.sync.dma_start(out=kt_sb[:, 1, :], in_=kt_d[1])
        nc.sync.dma_start(out=qt_sb[:, G, :], in_=qt_d[G])
        nc.scalar.dma_start(out=vp_sb[:, NT : 2 * NT, :], in_=vp_d[1].rearrange("j r c -> r j c"))
        for u in range(G + 1, NU):
            nc.sync.dma_start(out=qt_sb[:, u, :], in_=qt_d[u])

        PT_W = [128 * len(b) for b in CHUNK_BLOCKS]

        MA_BASE = {}
        _b = 0
        for _c in DVE_CHUNKS:
            MA_BASE[_c] = _b
            _b += PT_W[_c]

        def score_chunk(u, c):
            """Emit score matmuls + exp for chunk c of head u; returns pt."""
            p = u // G
            blks = CHUNK_BLOCKS[c]
            w = PT_W[c]
            ps = scp.tile([128, 1024], FP32, tag="sc", name=f"sc_{u}_{c}")
            pt = ptp.tile([128, w], FP16, tag=f"pt{c}", name=f"pt{c}_{u}")
            for idx, (j, i) in enumerate(blks):
                L = 128 * idx
                nc.tensor.matmul(
                    ps[:, L : L + 128],
                    lhsT=kt_sb[:, p, 128 * j : 128 * j + 128],
                    rhs=qt_sb[:, u, 128 * i : 128 * i + 128],
                    start=True, stop=True)
            if CHUNK_ENGINE[c] == "ACT":
                nc.scalar.activation(out=pt, in_=ps[:, 0:w], func=EXP, scale=SCALE)
                dw = DIAG_W[c]
                if dw:  # one batched 0/1 mask multiply over the diag blocks
                    nc.gpsimd.tensor_mul(pt[:, 0:dw], pt[:, 0:dw], dm_sb[:, 0:dw])
            else:
                mb = MA_BASE[c]
                nc.vector.scalar_tensor_tensor(
                    out=pt.bitcast(I16), in0=ps[:, 0:w], scalar=float(B16 / A16),
                    in1=ma_sb[:, mb : mb + w], op0=ADD, op1=MULT)
            return pt

        GROUP_ROWS = ((0, 1, 2), (3, 4, 5), (6, 7))

        def pv_group(u, g0, pts, ob):
            p = u // G
            rows = GROUP_ROWS[g0]
            po = pvp.tile([128, 3, VW], FP32, tag="pv", name=f"pv_{u}_{g0}")
            for r, i in enumerate(rows):
                for jj in range(i + 1):
                    c, L = BLOCK_POS[(jj, i)]
                    nc.tensor.matmul(po[:, r, :], lhsT=pts[c][:, L : L + 128],
                                     rhs=vp_sb[:, p * NT + jj, :],
                                     start=(jj == 0), stop=(jj == i))
            nr = len(rows)
            rec = recp.tile([128, 3], FP32, tag="rec", name=f"rec_{u}_{g0}")
            nc.vector.reciprocal_approx_fast(rec[:, 0:nr], po[:, 0:nr, D : D + 1])
            r0, r1 = rows[0], rows[-1] + 1
            nc.vector.scalar_tensor_tensor(
                out=ob[:, r0:r1, :], in0=po[:, 0:nr, 0:D], scalar=1.0,
                in1=rec[:, 0:nr].broadcast_to([128, nr, D]),
                op0=MULT, op1=MULT)
            r0, r1 = rows[0], rows[-1] + 1
            nc.sync.dma_start(out=o_d[u][:, r0:r1, :], in_=ob[:, r0:r1, :])

        # software pipeline at chunk/group granularity: head u's PV groups are
        # interleaved between head u+1's score chunks, and the final head's
        # groups are emitted as soon as their pt chunks exist
        pts_prev = [score_chunk(0, c) for c in range(5)]
        ob_prev = obp.tile([128, NT, D], FP16, tag="ob", name="ob_0")
        for u in range(1, NU):
            last = u == NU - 1
            pts = [None] * 5
            ob = obp.tile([128, NT, D], FP16, tag="ob", name=f"ob_{u}")
            pts[2] = score_chunk(u, 2)
            pts[0] = score_chunk(u, 0)
            pv_group(u - 1, 0, pts_prev, ob_prev)
            pts[1] = score_chunk(u, 1)
            pts[3] = score_chunk(u, 3)
            pv_group(u - 1, 1, pts_prev, ob_prev)
            pts[4] = score_chunk(u, 4)
            pv_group(u - 1, 2, pts_prev, ob_prev)
            pts_prev, ob_prev = pts, ob
        pv_group(NU - 1, 0, pts_prev, ob_prev)
        pv_group(NU - 1, 1, pts_prev, ob_prev)
        pv_group(NU - 1, 2, pts_prev, ob_prev)

    nc.compile()
    return nc


def _host_prep(q, k, v):
    """Per-core input maps (shard + transpose + fp16 cast on host)."""
    q16 = np.ascontiguousarray(q.astype(np.float16))
    k16 = np.ascontiguousarray(k.astype(np.float16))
    v16 = np.ascontiguousarray(v.astype(np.float16))

    ii = np.arange(128)
    diagm = (ii[:, None] <= ii[None, :]).astype(np.float32)  # [sk, sq] keep
    dm = np.tile(diagm.astype(np.float16), (1, 3))

    # DVE mask tile: A16 * [sk<=sq] on diagonal blocks, A16 elsewhere
    ma = np.ones((128, MA_W), np.float32)
    base = 0
    for c in DVE_CHUNKS:
        for idx, (j, i) in enumerate(CHUNK_BLOCKS[c]):
            if j == i:
                ma[:, base + 128 * idx : base + 128 * (idx + 1)] = diagm
        base += 128 * len(CHUNK_BLOCKS[c])
    ma = (ma * A16).astype(np.float16)

    in_maps = []
    for c in range(NCORES):
        qt = np.empty((NU, D, S), np.float16)
        kt = np.empty((PAIRS_PER_CORE, D, S), np.float16)
        vp = np.empty((PAIRS_PER_CORE, NT, 128, VW), np.float16)
        for p in range(PAIRS_PER_CORE):
            pg = c * PAIRS_PER_CORE + p
            b, g = divmod(pg, HKV)
            tok = slice(b * S, (b + 1) * S)
            kt[p] = k16[tok, g, :].T
            for hh in range(G):
                qt[p * G + hh] = q16[tok, g * G + hh, :].T
            vseg = v16[tok, g, :]
            vp[p, :, :, :D] = vseg.reshape(NT, 128, D)
            vp[p, :, :, D] = np.float16(1.0)
        in_maps.append({"qt": qt, "kt": kt, "vp": vp, "ma": ma, "dm": dm})
    return in_maps


def _gather(results):
    out = np.empty((B * S, H, D), np.float32)
    for c in range(NCORES):
        o = results[c]["o"]  # [NU, 128, NT, D] fp16
        of = np.asarray(o, np.float32).transpose(2, 1, 0, 3)  # [i, s, u, d]
        for p in range(PAIRS_PER_CORE):
            pg = c * PAIRS_PER_CORE + p
            b, g = divmod(pg, HKV)
            out[b * S : (b + 1) * S, g * G : (g + 1) * G, :] = of.reshape(
                S, NU, D)[:, p * G : (p + 1) * G, :]
    return out


def kernel(q, k, v, cu_seqlens_q=None, cu_seqlens_k=None, **_ignored):
    if "nc" not in _cache:
        _cache["nc"] = build_program()
    nc = _cache["nc"]
    in_maps = _host_prep(np.asarray(q), np.asarray(k), np.asarray(v))
    res = run_bass_kernel_spmd(nc, in_maps, core_ids=list(range(NCORES)))
    return _gather(res.results)
